# revision 6
# baseline (speedup 1.0000x reference)
"""CollectDiffuseAttention Trainium2 Bass kernel.

Reference computation (per batch b):
    cl[l]   = (q . kc[l]) / sqrt(D)                  # collect logits (also output 2)
    w       = softmax(cl)                            # over L
    col[d]  = sum_l w[l] * v[l, d]                   # collected value
    g[l]    = sigmoid((kd[l] . q) / sqrt(D))         # diffuse gate
    out[l,d]= g[l] * col[d]                          # outer product

Sharding: pure batch-parallel, 256 batches -> 32 per core on 8 cores.

Per-core kernel layout (per batch):
  - kc/kd/v streamed as [128, 4, 512] SBUF tiles (l on partitions, chunk c
    of 4 l-blocks, d on free dim) -> 1 MiB DMAs.
  - dot products on DVE via tensor_tensor_reduce against a partition-
    broadcast copy of q (scale folds in 1/sqrt(D)).
  - softmax without max-subtraction (logits are ~N(0,1); exp is safe):
    e = exp(cl) on ACT (accum_out gives per-partition sums), total S via a
    PE ones-matmul, normalization folded into the collected vector.
  - collected vector via PE: col = sum_c e[:,c]^T @ v_tile_c  (K=128, N=512).
  - sigmoid computed as 1/(1+exp(-x)) so ACT stays on the `exp` table set
    the whole kernel (sigmoid lives in a different set; switching costs
    ~2.7us each way).
  - outer product on ACT: out_tile[:, c, :] = colb * gate[:, c]  where colb
    is col broadcast to 128 partitions via a K=1 PE matmul.

Toolchain quirk: the pinned walrus accepts at most ONE semaphore wait and
ONE semaphore update per instruction; Tile freely emits more. We legalize
post-hoc by peeling excess waits/updates onto standalone InstNoOps.
"""

from contextlib import ExitStack

import numpy as np

import concourse.bass as bass
import concourse.tile as tile
from concourse import mybir
from concourse.bass_utils import run_bass_kernel_spmd

B, L, D = 256, 1024, 512
NCORES = 8
BPC = B // NCORES  # batches per core
TEMP = float(np.sqrt(float(D)))
P = 128
NCH = L // P       # 8 l-chunks of 128
CPB = 4            # l-chunks per DMA tile
NBIG = NCH // CPB  # 2 big tiles per tensor per batch
F32 = mybir.dt.float32

_LEG_CTR = [0]


def _legalize_single_wait(nc, max_waits=1, max_updates=1):
    """Peel excess sem waits (before, same engine) and updates (after) onto
    standalone InstNoOps: this walrus build allows only one of each per
    instruction."""
    for f in nc.m.functions:
        for b in f.blocks:
            insts = b.instructions  # live list
            out = []
            for inst in insts:
                si = inst.sync_info
                pre, post = [], []
                if si is not None and (
                    len(si.on_wait) > max_waits or len(si.on_update) > max_updates
                ):
                    waits = list(si.on_wait)
                    upds = list(si.on_update)
                    if len(waits) > max_waits:
                        extra_w, keep_w = waits[:-max_waits], waits[-max_waits:]
                    else:
                        extra_w, keep_w = [], waits
                    keep_u, extra_u = upds[:max_updates], upds[max_updates:]
                    for w in extra_w:
                        _LEG_CTR[0] += 1
                        pre.append(
                            mybir.InstNoOp(
                                name=f"I-legw-{_LEG_CTR[0]}",
                                sync_info=mybir.SyncInfo(on_wait=[w], on_update=[]),
                                bass_nofuse=True,
                                engine=inst.engine,
                            )
                        )
                    for u in extra_u:
                        _LEG_CTR[0] += 1
                        post.append(
                            mybir.InstNoOp(
                                name=f"I-legu-{_LEG_CTR[0]}",
                                sync_info=mybir.SyncInfo(on_wait=[], on_update=[u]),
                                bass_nofuse=True,
                                engine=inst.engine,
                            )
                        )
                    inst.sync_info = mybir.SyncInfo(on_wait=keep_w, on_update=keep_u)
                out.extend(pre)
                out.append(inst)
                out.extend(post)
            insts[:] = out


def _build(bpc=BPC):
    nc = bass.Bass()
    q_h = nc.dram_tensor("q", [bpc, 1, D], F32, kind="ExternalInput")
    kc_h = nc.dram_tensor("kc", [bpc, L, D], F32, kind="ExternalInput")
    kd_h = nc.dram_tensor("kd", [bpc, L, D], F32, kind="ExternalInput")
    v_h = nc.dram_tensor("v", [bpc, L, D], F32, kind="ExternalInput")
    out_h = nc.dram_tensor("out", [bpc, L, D], F32, kind="ExternalOutput")
    lg_h = nc.dram_tensor("lg", [bpc, L], F32, kind="ExternalOutput")

    mult = mybir.AluOpType.mult
    add = mybir.AluOpType.add
    EXP = mybir.ActivationFunctionType.Exp

    with tile.TileContext(nc) as tc, ExitStack() as ctx:
        consts = ctx.enter_context(tc.tile_pool(name="consts", bufs=1))
        kcp = ctx.enter_context(tc.tile_pool(name="kcp", bufs=3))
        kdp = ctx.enter_context(tc.tile_pool(name="kdp", bufs=3))
        vp = ctx.enter_context(tc.tile_pool(name="vp", bufs=3))
        op = ctx.enter_context(tc.tile_pool(name="op", bufs=4))
        qp = ctx.enter_context(tc.tile_pool(name="qp", bufs=2))
        scp = ctx.enter_context(tc.tile_pool(name="scp", bufs=2))
        sm = ctx.enter_context(tc.tile_pool(name="sm", bufs=3))
        psS = ctx.enter_context(tc.tile_pool(name="psS", bufs=2, space="PSUM"))
        psC = ctx.enter_context(tc.tile_pool(name="psC", bufs=2, space="PSUM"))
        psB = ctx.enter_context(tc.tile_pool(name="psB", bufs=2, space="PSUM"))

        ones_col = consts.tile([P, 1], F32)
        nc.vector.memset(ones_col, 1.0)
        ones_row = consts.tile([1, P], F32)
        nc.vector.memset(ones_row, 1.0)

        for b in range(bpc):
            kc_r = kc_h[b].rearrange("(c p) d -> p c d", p=P)
            kd_r = kd_h[b].rearrange("(c p) d -> p c d", p=P)
            v_r = v_h[b].rearrange("(c p) d -> p c d", p=P)
            out_r = out_h[b].rearrange("(c p) d -> p c d", p=P)

            qb = qp.tile([P, D], F32)
            nc.gpsimd.dma_start(out=qb, in_=q_h[b, 0:1, :].broadcast_to([P, D]))

            kct = []
            for j in range(NBIG):
                t = kcp.tile([P, CPB, D], F32, tag="kct")
                nc.sync.dma_start(out=t, in_=kc_r[:, j * CPB : (j + 1) * CPB, :])
                kct.append(t)

            # collect logits: cl[p, c] = (kc[l] . q) / T,  l = c*128 + p
            scratch = scp.tile([P, D], F32)
            cl = sm.tile([P, NCH], F32)
            for c in range(NCH):
                nc.vector.scalar_tensor_tensor(
                    out=scratch,
                    in0=kct[c // CPB][:, c % CPB, :],
                    scalar=1.0 / TEMP,
                    in1=qb,
                    op0=mult,
                    op1=mult,
                    accum_out=cl[:, c : c + 1],
                )
            nc.sync.dma_start(out=lg_h[b].rearrange("(c p) -> p c", p=P), in_=cl)

            # e = exp(cl), esum[p] = sum_c e[p, c]
            e = sm.tile([P, NCH], F32)
            esum = sm.tile([P, 1], F32)
            nc.scalar.activation(out=e, in_=cl, func=EXP, accum_out=esum)

            # S = sum_p esum[p]  (PE cross-partition sum), recipS = 1/S
            S = psS.tile([1, 1], F32)
            nc.tensor.matmul(S, lhsT=esum, rhs=ones_col, start=True, stop=True)
            recipS = sm.tile([1, 1], F32)
            nc.vector.reciprocal(recipS, S)

            # collected value: colU[d] = sum_l e[l] * v[l, d]
            vt = []
            for j in range(NBIG):
                t = vp.tile([P, CPB, D], F32, tag="vt")
                nc.sync.dma_start(out=t, in_=v_r[:, j * CPB : (j + 1) * CPB, :])
                vt.append(t)
            colU = psC.tile([1, D], F32)
            for c in range(NCH):
                nc.tensor.matmul(
                    colU,
                    lhsT=e[:, c : c + 1],
                    rhs=vt[c // CPB][:, c % CPB, :],
                    start=(c == 0),
                    stop=(c == NCH - 1),
                )
            # colS = colU / S ; broadcast to 128 partitions via K=1 matmul
            colS = sm.tile([1, D], F32)
            nc.scalar.mul(colS, colU, recipS)
            colb = psB.tile([P, D], F32)
            nc.tensor.matmul(colb, lhsT=ones_row, rhs=colS, start=True, stop=True)

            # diffuse gate: gl[p, c] = -(kd[l] . q) / T ; g = 1/(1+exp(gl))
            kdt = []
            for j in range(NBIG):
                t = kdp.tile([P, CPB, D], F32, tag="kdt")
                nc.sync.dma_start(out=t, in_=kd_r[:, j * CPB : (j + 1) * CPB, :])
                kdt.append(t)
            gl = sm.tile([P, NCH], F32)
            for c in range(NCH):
                nc.vector.scalar_tensor_tensor(
                    out=scratch,
                    in0=kdt[c // CPB][:, c % CPB, :],
                    scalar=-1.0 / TEMP,
                    in1=qb,
                    op0=mult,
                    op1=mult,
                    accum_out=gl[:, c : c + 1],
                )
            eg = sm.tile([P, NCH], F32)
            nc.scalar.activation(out=eg, in_=gl, func=EXP)  # exp(-logit)
            egp = sm.tile([P, NCH], F32)
            nc.vector.tensor_scalar_add(egp, eg, 1.0)
            g = sm.tile([P, NCH], F32)
            nc.vector.reciprocal(g, egp)

            # out[l, d] = g[l] * col[d]
            for j in range(NBIG):
                ot = op.tile([P, CPB, D], F32, tag="ot")
                for cc in range(CPB):
                    c = j * CPB + cc
                    nc.scalar.mul(ot[:, cc, :], colb, g[:, c : c + 1])
                nc.sync.dma_start(out=out_r[:, j * CPB : (j + 1) * CPB, :], in_=ot)

    _legalize_single_wait(nc)
    return nc


_NC_CACHE = {}


def _get_nc():
    if "nc" not in _NC_CACHE:
        _NC_CACHE["nc"] = _build()
    return _NC_CACHE["nc"]


def kernel(q, kc, kd, v):
    q = np.ascontiguousarray(q, dtype=np.float32)
    kc = np.ascontiguousarray(kc, dtype=np.float32)
    kd = np.ascontiguousarray(kd, dtype=np.float32)
    v = np.ascontiguousarray(v, dtype=np.float32)
    nc = _get_nc()
    in_maps = []
    for i in range(NCORES):
        s = slice(i * BPC, (i + 1) * BPC)
        in_maps.append({"q": q[s], "kc": kc[s], "kd": kd[s], "v": v[s]})
    res = run_bass_kernel_spmd(nc, in_maps, core_ids=list(range(NCORES)))
    out = np.concatenate([r["out"] for r in res.results], axis=0)
    lg = np.concatenate([r["lg"] for r in res.results], axis=0)
    return out, lg


# revision 8
# speedup vs baseline: 28.4061x; 28.4061x over previous
"""CollectDiffuseAttention Trainium2 Bass kernel.

Reference computation (per batch b):
    cl[l]   = (q . kc[l]) / sqrt(D)                  # collect logits (also output 2)
    w       = softmax(cl)                            # over L
    col[d]  = sum_l w[l] * v[l, d]                   # collected value
    g[l]    = sigmoid((kd[l] . q) / sqrt(D))         # diffuse gate
    out[l,d]= g[l] * col[d]                          # outer product

Sharding: pure batch-parallel, 256 batches -> 32 per core on 8 cores.

Per-core kernel layout (per batch):
  - kc/kd/v streamed as [128, 4, 512] SBUF tiles (l on partitions, chunk c
    of 4 l-blocks, d on free dim) -> 1 MiB DMAs.
  - dot products on DVE via tensor_tensor_reduce against a partition-
    broadcast copy of q (scale folds in 1/sqrt(D)).
  - softmax without max-subtraction (logits are ~N(0,1); exp is safe):
    e = exp(cl) on ACT (accum_out gives per-partition sums), total S via a
    PE ones-matmul, normalization folded into the collected vector.
  - collected vector via PE: col = sum_c e[:,c]^T @ v_tile_c  (K=128, N=512).
  - sigmoid computed as 1/(1+exp(-x)) so ACT stays on the `exp` table set
    the whole kernel (sigmoid lives in a different set; switching costs
    ~2.7us each way).
  - outer product on ACT: out_tile[:, c, :] = colb * gate[:, c]  where colb
    is col broadcast to 128 partitions via a K=1 PE matmul.

Toolchain quirk: the pinned walrus accepts at most ONE semaphore wait and
ONE semaphore update per instruction; Tile freely emits more. We legalize
post-hoc by peeling excess waits/updates onto standalone InstNoOps.
"""

from contextlib import ExitStack

import numpy as np

import concourse.bass as bass
import concourse.tile as tile
from concourse import mybir
from concourse.bass_utils import run_bass_kernel_spmd

B, L, D = 256, 1024, 512
NCORES = 8
BPC = B // NCORES  # batches per core
TEMP = float(np.sqrt(float(D)))
P = 128
NCH = L // P       # 8 l-chunks of 128
CPB = 4            # l-chunks per DMA tile
NBIG = NCH // CPB  # 2 big tiles per tensor per batch
F32 = mybir.dt.float32

_LEG_CTR = [0]


def _legalize_single_wait(nc, max_waits=1, max_updates=1):
    """Peel excess sem waits (before, same engine) and updates (after) onto
    standalone InstNoOps: this walrus build allows only one of each per
    instruction."""
    for f in nc.m.functions:
        for b in f.blocks:
            insts = b.instructions  # live list
            out = []
            for inst in insts:
                si = inst.sync_info
                pre, post = [], []
                if si is not None and (
                    len(si.on_wait) > max_waits or len(si.on_update) > max_updates
                ):
                    waits = list(si.on_wait)
                    upds = list(si.on_update)
                    if len(waits) > max_waits:
                        extra_w, keep_w = waits[:-max_waits], waits[-max_waits:]
                    else:
                        extra_w, keep_w = [], waits
                    keep_u, extra_u = upds[:max_updates], upds[max_updates:]
                    for w in extra_w:
                        _LEG_CTR[0] += 1
                        pre.append(
                            mybir.InstNoOp(
                                name=f"I-legw-{_LEG_CTR[0]}",
                                sync_info=mybir.SyncInfo(on_wait=[w], on_update=[]),
                                bass_nofuse=True,
                                engine=inst.engine,
                            )
                        )
                    for u in extra_u:
                        _LEG_CTR[0] += 1
                        post.append(
                            mybir.InstNoOp(
                                name=f"I-legu-{_LEG_CTR[0]}",
                                sync_info=mybir.SyncInfo(on_wait=[], on_update=[u]),
                                bass_nofuse=True,
                                engine=inst.engine,
                            )
                        )
                    inst.sync_info = mybir.SyncInfo(on_wait=keep_w, on_update=keep_u)
                out.extend(pre)
                out.append(inst)
                out.extend(post)
            insts[:] = out


def _build(bpc=BPC, repeats=1):
    nc = bass.Bass()
    q_h = nc.dram_tensor("q", [bpc, 1, D], F32, kind="ExternalInput")
    kc_h = nc.dram_tensor("kc", [bpc, L, D], F32, kind="ExternalInput")
    kd_h = nc.dram_tensor("kd", [bpc, L, D], F32, kind="ExternalInput")
    v_h = nc.dram_tensor("v", [bpc, L, D], F32, kind="ExternalInput")
    out_h = nc.dram_tensor("out", [bpc, L, D], F32, kind="ExternalOutput")
    lg_h = nc.dram_tensor("lg", [bpc, L], F32, kind="ExternalOutput")

    mult = mybir.AluOpType.mult
    add = mybir.AluOpType.add
    EXP = mybir.ActivationFunctionType.Exp

    with tile.TileContext(nc) as tc, ExitStack() as ctx:
        consts = ctx.enter_context(tc.tile_pool(name="consts", bufs=1))
        kcp = ctx.enter_context(tc.tile_pool(name="kcp", bufs=3))
        kdp = ctx.enter_context(tc.tile_pool(name="kdp", bufs=3))
        vp = ctx.enter_context(tc.tile_pool(name="vp", bufs=3))
        op = ctx.enter_context(tc.tile_pool(name="op", bufs=4))
        qp = ctx.enter_context(tc.tile_pool(name="qp", bufs=2))
        scp = ctx.enter_context(tc.tile_pool(name="scp", bufs=2))
        sm = ctx.enter_context(tc.tile_pool(name="sm", bufs=3))
        psS = ctx.enter_context(tc.tile_pool(name="psS", bufs=2, space="PSUM"))
        psC = ctx.enter_context(tc.tile_pool(name="psC", bufs=2, space="PSUM"))
        psB = ctx.enter_context(tc.tile_pool(name="psB", bufs=2, space="PSUM"))

        ones_col = consts.tile([P, 1], F32)
        nc.vector.memset(ones_col, 1.0)
        ones_row = consts.tile([1, P], F32)
        nc.vector.memset(ones_row, 1.0)

        for b in [bb for _ in range(repeats) for bb in range(bpc)]:
            kc_r = kc_h[b].rearrange("(c p) d -> p c d", p=P)
            kd_r = kd_h[b].rearrange("(c p) d -> p c d", p=P)
            v_r = v_h[b].rearrange("(c p) d -> p c d", p=P)
            out_r = out_h[b].rearrange("(c p) d -> p c d", p=P)

            qb = qp.tile([P, D], F32)
            nc.gpsimd.dma_start(out=qb, in_=q_h[b, 0:1, :].broadcast_to([P, D]))

            kct = []
            for j in range(NBIG):
                t = kcp.tile([P, CPB, D], F32, tag="kct")
                nc.sync.dma_start(out=t, in_=kc_r[:, j * CPB : (j + 1) * CPB, :])
                kct.append(t)

            # collect logits: cl[p, c] = (kc[l] . q) / T,  l = c*128 + p
            scratch = scp.tile([P, D], F32)
            cl = sm.tile([P, NCH], F32)
            for c in range(NCH):
                nc.vector.scalar_tensor_tensor(
                    out=scratch,
                    in0=kct[c // CPB][:, c % CPB, :],
                    scalar=1.0 / TEMP,
                    in1=qb,
                    op0=mult,
                    op1=mult,
                    accum_out=cl[:, c : c + 1],
                )
            nc.sync.dma_start(out=lg_h[b].rearrange("(c p) -> p c", p=P), in_=cl)

            # e = exp(cl), esum[p] = sum_c e[p, c]
            e = sm.tile([P, NCH], F32)
            esum = sm.tile([P, 1], F32)
            nc.scalar.activation(out=e, in_=cl, func=EXP, accum_out=esum)

            # S = sum_p esum[p]  (PE cross-partition sum), recipS = 1/S
            S = psS.tile([1, 1], F32)
            nc.tensor.matmul(S, lhsT=esum, rhs=ones_col, start=True, stop=True)
            recipS = sm.tile([1, 1], F32)
            nc.vector.reciprocal(recipS, S)

            # collected value: colU[d] = sum_l e[l] * v[l, d]
            vt = []
            for j in range(NBIG):
                t = vp.tile([P, CPB, D], F32, tag="vt")
                nc.sync.dma_start(out=t, in_=v_r[:, j * CPB : (j + 1) * CPB, :])
                vt.append(t)
            colU = psC.tile([1, D], F32)
            for c in range(NCH):
                nc.tensor.matmul(
                    colU,
                    lhsT=e[:, c : c + 1],
                    rhs=vt[c // CPB][:, c % CPB, :],
                    start=(c == 0),
                    stop=(c == NCH - 1),
                )
            # colS = colU / S ; broadcast to 128 partitions via K=1 matmul
            colS = sm.tile([1, D], F32)
            nc.scalar.mul(colS, colU, recipS)
            colb = psB.tile([P, D], F32)
            nc.tensor.matmul(colb, lhsT=ones_row, rhs=colS, start=True, stop=True)

            # diffuse gate: gl[p, c] = -(kd[l] . q) / T ; g = 1/(1+exp(gl))
            kdt = []
            for j in range(NBIG):
                t = kdp.tile([P, CPB, D], F32, tag="kdt")
                nc.sync.dma_start(out=t, in_=kd_r[:, j * CPB : (j + 1) * CPB, :])
                kdt.append(t)
            gl = sm.tile([P, NCH], F32)
            for c in range(NCH):
                nc.vector.scalar_tensor_tensor(
                    out=scratch,
                    in0=kdt[c // CPB][:, c % CPB, :],
                    scalar=-1.0 / TEMP,
                    in1=qb,
                    op0=mult,
                    op1=mult,
                    accum_out=gl[:, c : c + 1],
                )
            eg = sm.tile([P, NCH], F32)
            nc.scalar.activation(out=eg, in_=gl, func=EXP)  # exp(-logit)
            egp = sm.tile([P, NCH], F32)
            nc.vector.tensor_scalar_add(egp, eg, 1.0)
            g = sm.tile([P, NCH], F32)
            nc.vector.reciprocal(g, egp)

            # out[l, d] = g[l] * col[d]
            for j in range(NBIG):
                ot = op.tile([P, CPB, D], F32, tag="ot")
                for cc in range(CPB):
                    c = j * CPB + cc
                    nc.scalar.mul(ot[:, cc, :], colb, g[:, c : c + 1])
                nc.sync.dma_start(out=out_r[:, j * CPB : (j + 1) * CPB, :], in_=ot)

    _legalize_single_wait(nc)
    return nc


_NC_CACHE = {}


def _get_nc():
    if "nc" not in _NC_CACHE:
        _NC_CACHE["nc"] = _build()
    return _NC_CACHE["nc"]


def kernel(q, kc, kd, v):
    q = np.ascontiguousarray(q, dtype=np.float32)
    kc = np.ascontiguousarray(kc, dtype=np.float32)
    kd = np.ascontiguousarray(kd, dtype=np.float32)
    v = np.ascontiguousarray(v, dtype=np.float32)
    nc = _get_nc()
    in_maps = []
    for i in range(NCORES):
        s = slice(i * BPC, (i + 1) * BPC)
        in_maps.append({"q": q[s], "kc": kc[s], "kd": kd[s], "v": v[s]})
    res = run_bass_kernel_spmd(nc, in_maps, core_ids=list(range(NCORES)))
    out = np.concatenate([r["out"] for r in res.results], axis=0)
    lg = np.concatenate([r["lg"] for r in res.results], axis=0)
    return out, lg
